# revision 5
# baseline (speedup 1.0000x reference)
"""Distributed Trainium2 kernel for nn_DecoderAttentionRotary.

Strategy (8 NeuronCores, tensor-parallel over heads, fp16 matmul datapath):
  - host: transpose x -> xT [D, B*L] fp16; per-core Wqkv column slice
    reordered to [q0,k0,q1,k1,v0|v1] fp16; cos/sin transposed (single
    batch) fp16; one 128x128 causal mask fp16.
  - device, per core (2 heads):
      phase 1 (both batches, continuous xT stream): qkT = (Wqk^T @ xT) + b
               (fp16 matmuls, fp32 psum, bias via ACT), v = x @ Wv in
               [l, hd] layout; RoPE fused per 512-column chunk (pure DVE,
               no shuffle DMA); Wd streamed into SBUF during phase 1.
      phase 2: causal attention in scores^T layout, software-pipelined
               depth-2 on PE: score(ki+2) issues before out(ki) so the
               exp (ACT) latency is hidden; row-sums accumulated on DVE
               (sumacc += et) instead of ones-matmuls on PE; per-qc
               partition_all_reduce (gpsimd) + reciprocal + mult.
      per-(batch,head) AllToAll reshard (fp16, 4 small collectives) so
      only the last 0.5MB collective is exposed; o_sb loads are plain
      [128,256] copies on the gpsimd queue.
      phase 3: y rows = o^T @ Wd + bd from the SBUF-resident Wd.
  - host: scatter the per-core 256-row halves into the full output.
"""
import sys

for _p in ("/opt/pypackages", "/opt/trn_rl_repo"):
    if _p not in sys.path:
        sys.path.insert(0, _p)

import numpy as np

B, L, D, H = 2, 2048, 2048, 16
HD, R = 128, 32
SCALE = float(HD) ** -0.5
W = 8
HPC = H // W              # heads per core
M = B * L                 # flattened rows
CORES = list(range(W))

_NC = None


def _build_nc():
    import concourse.mybir as mybir
    import concourse.tile as tile
    from concourse import bacc, bass_isa

    f32 = mybir.dt.float32
    f16 = mybir.dt.float16
    AFT = mybir.ActivationFunctionType
    OP = mybir.AluOpType

    nc = bacc.Bacc(None, target_bir_lowering=False, num_devices=W)
    xT = nc.declare_dram_parameter("xT", [D, M], f16, isOutput=False)
    wqkv = nc.declare_dram_parameter("wqkv", [D, 6 * HD], f16, isOutput=False)
    bqk = nc.declare_dram_parameter("bqk", [4 * HD, 1], f32, isOutput=False)
    bv = nc.declare_dram_parameter("bv", [1, 2 * HD], f16, isOutput=False)
    cosT = nc.declare_dram_parameter("cosT", [R, L], f16, isOutput=False)
    sinT = nc.declare_dram_parameter("sinT", [R, L], f16, isOutput=False)
    mask0 = nc.declare_dram_parameter("mask0", [128, 128], f16, isOutput=False)
    wd = nc.declare_dram_parameter("wd", [D, D], f16, isOutput=False)
    bdb = nc.declare_dram_parameter("bdb", [128, D], f16, isOutput=False)
    y = nc.declare_dram_parameter("y", [M // W, D], f32, isOutput=True)

    xT_r = xT.ap().rearrange("(t p) n -> p t n", p=128)   # [128, 16, M]
    wq_r = wqkv.ap().rearrange("(t p) m -> p t m", p=128)
    wd_r = wd.ap().rearrange("(t p) n -> p t n", p=128)

    with tile.TileContext(nc) as tc:
        with (
            tc.tile_pool(name="const", bufs=1) as cpool,
            tc.tile_pool(name="dram", bufs=1, space="DRAM") as dpool,
            tc.tile_pool(name="ps", bufs=1, space="PSUM") as pp,
            tc.tile_pool(name="qkv", bufs=1) as qkvpool,
            tc.tile_pool(name="p3s", bufs=2) as p3s,
            tc.tile_pool(name="att", bufs=2) as apool,
            tc.tile_pool(name="p1", bufs=2) as p1pool,
        ):
            a2a_ins = [[dpool.tile([W, HD, 256], f16, name=f"a2ain{b}_{h}")
                        for h in range(HPC)] for b in range(B)]
            a2a_outs = [[dpool.tile([W, HD, 256], f16, name=f"a2aout{b}_{h}")
                         for h in range(HPC)] for b in range(B)]

            w_sb = cpool.tile([128, 16, 6 * HD], f16)
            wd_sb = cpool.tile([128, 16, D], f16)
            bd_sb = cpool.tile([128, D], f16)
            bqk_sb = cpool.tile([128, 4], f32)
            bv_sb = cpool.tile([1, 2 * HD], f16)
            ones_r = cpool.tile([1, 128], f16)
            cos_sb = cpool.tile([R, L], f16)
            sin_sb = cpool.tile([R, L], f16)
            mask_sb = cpool.tile([128, 128], f16)
            qk_sbs, v_sbs, o_sbs = [], [], []
            for b in range(B):
                qk_sbs.append(qkvpool.tile([128, 4, L], f16, name=f"qk{b}"))
                v_sbs.append(qkvpool.tile([128, 16, 2 * HD], f16, name=f"v{b}"))
                o_sbs.append(qkvpool.tile([128, 16, 256], f16, name=f"osb{b}"))

            # ---- startup DMA ordering: chunk-0 x tiles first, then weights
            xt_store = {}

            def load_xt(ci):
                b, nch = divmod(ci, 4)
                n0 = b * L + nch * 512
                tiles = []
                for half in range(2):
                    xt = p1pool.tile([128, 8, 512], f16, tag="xt", bufs=3,
                                     name=f"xt{ci}_{half}")
                    for piece in range(2):
                        t0 = half * 8 + piece * 4
                        nc.sync.dma_start(
                            out=xt[:, piece * 4:(piece + 1) * 4, :],
                            in_=xT_r[:, t0:t0 + 4, n0:n0 + 512],
                        )
                    tiles.append(xt)
                xt_store[ci] = tiles

            load_xt(0)
            for wq in range(8):
                nc.sync.dma_start(
                    out=w_sb[:, 2 * wq:2 * (wq + 1), :],
                    in_=wq_r[:, 2 * wq:2 * (wq + 1), :],
                )
            nc.sync.dma_start(
                out=bqk_sb[:], in_=bqk.ap().rearrange("(t p) o -> p (t o)", p=128)
            )
            nc.sync.dma_start(out=bv_sb[:], in_=bv.ap())
            nc.vector.memset(ones_r[:], 1.0)

            # ---- phase 1 (both batches) + fused RoPE ----
            for ci in range(2 * (L // 512)):
                b, nch = divmod(ci, 4)
                qk_sb, v_sb = qk_sbs[b], v_sbs[b]
                n0 = b * L + nch * 512
                ch = slice(nch * 512, (nch + 1) * 512)
                if ci + 1 < 2 * (L // 512):
                    load_xt(ci + 1)
                if ci == 0:
                    # lower-priority constants after chunk-0/1 x tiles
                    nc.sync.dma_start(out=cos_sb[:], in_=cosT.ap())
                    nc.sync.dma_start(out=sin_sb[:], in_=sinT.ap())
                    nc.sync.dma_start(out=mask_sb[:], in_=mask0.ap())
                    nc.gpsimd.dma_start(out=bd_sb[:], in_=bdb.ap())
                xt_tiles = xt_store.pop(ci)
                for mp in range(2):
                    pss = [
                        pp.tile([128, 512], f32, tag="work", bufs=4,
                                name=f"qkps{ci}_{2 * mp + i}")
                        for i in range(2)
                    ]
                    for kt in range(16):
                        xt = xt_tiles[kt // 8]
                        for i in range(2):
                            m = 2 * mp + i
                            nc.tensor.matmul(
                                pss[i][:],
                                lhsT=w_sb[:, kt, m * 128:(m + 1) * 128],
                                rhs=xt[:, kt % 8, :],
                                start=(kt == 0),
                                stop=(kt == 15),
                            )
                    for i in range(2):
                        m = 2 * mp + i
                        nc.scalar.activation(
                            qk_sb[:, m, ch], pss[i][:], AFT.Identity,
                            bias=bqk_sb[:, m:m + 1],
                        )
                for m in range(4):
                    # fused RoPE on rows 0:R of this chunk: shuffle via DMA
                    # (DVE cannot shift partitions), then 3 in-place DVE ops
                    cs = cos_sb[:, ch]
                    sn = sin_sb[:, ch]
                    rot = p1pool.tile([R, 512], f16, tag="rot", bufs=2)
                    nc.gpsimd.dma_start(out=rot[0:16, :], in_=qk_sb[16:32, m, ch])
                    nc.gpsimd.dma_start(out=rot[16:32, :], in_=qk_sb[0:16, m, ch])
                    nc.vector.tensor_tensor(
                        qk_sb[0:R, m, ch], qk_sb[0:R, m, ch], cs, op=OP.mult
                    )
                    nc.vector.tensor_tensor(rot[:], rot[:], sn, op=OP.mult)
                    nc.vector.tensor_tensor(
                        qk_sb[0:R, m, ch], qk_sb[0:R, m, ch], rot[:], op=OP.add
                    )
                for rr2 in range(2):
                    vpss = [
                        pp.tile([128, 2 * HD], f32, tag="acc", bufs=3,
                                name=f"vps{ci}_{2 * rr2 + i}")
                        for i in range(2)
                    ]
                    for kt in range(16):
                        xt = xt_tiles[kt // 8]
                        for i in range(2):
                            rr = 2 * rr2 + i
                            nc.tensor.matmul(
                                vpss[i][:],
                                lhsT=xt[:, kt % 8, rr * 128:(rr + 1) * 128],
                                rhs=w_sb[:, kt, 4 * HD:6 * HD],
                                start=(kt == 0),
                                stop=False,
                            )
                    for i in range(2):
                        rr = 2 * rr2 + i
                        nc.tensor.matmul(
                            vpss[i][:], lhsT=ones_r[:], rhs=bv_sb[:],
                            start=False, stop=True,
                        )
                        nc.scalar.activation(
                            v_sb[:, nch * 4 + rr, :], vpss[i][:], AFT.Copy
                        )
                # stream Wd into SBUF behind the x tiles
                nc.sync.dma_start(
                    out=wd_sb[:, 2 * ci:2 * (ci + 1), :],
                    in_=wd_r[:, 2 * ci:2 * (ci + 1), :],
                )

            # ---- phase 2: attention; per-(b,h) A2A ----
            for b in range(B):
                qk_sb, v_sb = qk_sbs[b], v_sbs[b]
                for h in range(HPC):
                    for qc in reversed(range(L // 512)):
                        nk = 4 * qc + 4
                        outp = pp.tile([128, 512], f32, tag="acc", bufs=3,
                                       name=f"outp{b}_{h}_{qc}")
                        sumacc = apool.tile([128, 512], f16, tag="sumacc",
                                            bufs=2)
                        nc.vector.memset(sumacc[:], 0.0)

                        def emit_out(ki, et, c0, npr):
                            nc.tensor.matmul(
                                outp[:, c0:512],
                                lhsT=v_sb[:, ki, h * 128:(h + 1) * 128],
                                rhs=et[:, 0:npr],
                                start=(ki == 0), stop=(ki == nk - 1),
                            )

                        pend = []
                        for ki in range(nk):
                            # causal: diagonal k-tiles only q-cols >= j*128
                            j = max(0, ki - qc * 4)
                            c0 = j * 128
                            npr = 512 - c0
                            qs = slice(qc * 512 + c0, (qc + 1) * 512)
                            sp = pp.tile([128, 512], f32, tag="work", bufs=4,
                                         name=f"sp{b}_{h}_{qc}_{ki}")
                            nc.tensor.matmul(
                                sp[:, 0:npr],
                                lhsT=qk_sb[:, 2 * h + 1, ki * 128:(ki + 1) * 128],
                                rhs=qk_sb[:, 2 * h, qs],
                                start=True, stop=True,
                            )
                            et = apool.tile([128, 512], f16, tag="et", bufs=3)
                            nc.scalar.activation(
                                et[:, 0:npr], sp[:, 0:npr], AFT.Exp, scale=SCALE
                            )
                            if ki >= qc * 4:
                                nc.vector.tensor_tensor(
                                    et[:, 0:128], et[:, 0:128], mask_sb[:],
                                    op=OP.mult,
                                )
                            nc.vector.tensor_tensor(
                                sumacc[:, c0:512], sumacc[:, c0:512],
                                et[:, 0:npr], op=OP.add,
                            )
                            pend.append((ki, et, c0, npr))
                            if len(pend) > 2:
                                emit_out(*pend.pop(0))
                        for args_ in pend:
                            emit_out(*args_)
                        bcs = apool.tile([128, 512], f16, tag="bcs", bufs=2)
                        nc.gpsimd.partition_all_reduce(
                            bcs[:], sumacc[:], 128, bass_isa.ReduceOp.add
                        )
                        rec = apool.tile([128, 512], f16, tag="rec", bufs=2)
                        with nc.allow_low_precision("softmax denom fp16 ok"):
                            nc.vector.reciprocal(rec[:], bcs[:])
                        ot = apool.tile([128, 512], f16, tag="ot", bufs=2)
                        nc.vector.tensor_tensor(
                            ot[:], outp[:], rec[:], op=OP.mult
                        )
                        for half in range(2):
                            nc.sync.dma_start(
                                out=a2a_ins[b][h][2 * qc + half, :, :],
                                in_=ot[:, half * 256:(half + 1) * 256],
                            )
                    nc.gpsimd.collective_compute(
                        "AllToAll",
                        mybir.AluOpType.bypass,
                        replica_groups=[CORES],
                        ins=[a2a_ins[b][h][:]],
                        outs=[a2a_outs[b][h][:]],
                    )
                    # resharded half: plain [128,256] copies, gpsimd queue
                    for jsrc in range(W):
                        nc.gpsimd.dma_start(
                            out=o_sbs[b][:, 2 * jsrc + h, :],
                            in_=a2a_outs[b][h][jsrc, :, :],
                        )

            # ---- phase 3: output projection, b=0 half then b=1 half ----
            for bh in range(2):
                for n4 in range(4):
                    for i in range(2):
                        m = 2 * bh + i
                        yp = pp.tile([128, 512], f32,
                                     tag=("work" if i else "acc"),
                                     bufs=(4 if i else 3),
                                     name=f"yps{n4}_{m}")
                        for kt in range(16):
                            nc.tensor.matmul(
                                yp[:],
                                lhsT=o_sbs[bh][:, kt, i * 128:(i + 1) * 128],
                                rhs=wd_sb[:, kt, n4 * 512:(n4 + 1) * 512],
                                start=(kt == 0), stop=(kt == 15),
                            )
                        yt = p3s.tile([128, 512], f32, tag="yt")
                        nc.vector.tensor_tensor(
                            yt[:], yp[:], bd_sb[:, n4 * 512:(n4 + 1) * 512],
                            op=OP.add,
                        )
                        nc.sync.dma_start(
                            out=y[m * 128:(m + 1) * 128,
                                  n4 * 512:(n4 + 1) * 512],
                            in_=yt[:],
                        )
    nc.finalize()
    return nc


def _host_prep(x_BLD, cos, sin, Wqkv, bqkv, Wd, bd):
    x = np.asarray(x_BLD, np.float32).reshape(M, D)
    xT = np.ascontiguousarray(x.T.astype(np.float16))
    cosT = np.ascontiguousarray(
        np.asarray(cos, np.float32).reshape(L, R).T.astype(np.float16)
    )
    s2 = np.asarray(sin, np.float32).reshape(L, R).T
    sinT_pm = np.ascontiguousarray(
        np.concatenate([-s2[:16], s2[16:]], axis=0).astype(np.float16)
    )
    kk = np.arange(128, dtype=np.int64)[:, None]
    qq = np.arange(128, dtype=np.int64)[None, :]
    mask0 = np.ascontiguousarray((qq >= kk).astype(np.float16))
    bdb = np.ascontiguousarray(
        np.broadcast_to(np.asarray(bd, np.float32).astype(np.float16), (128, D))
    )
    Wqkv = np.asarray(Wqkv, np.float32)
    bqkv = np.asarray(bqkv, np.float32)
    in_maps = []
    for c in range(W):
        base = c * HPC * 3 * HD
        qk_idx = np.concatenate(
            [np.arange(base + h * 3 * HD, base + h * 3 * HD + 2 * HD)
             for h in range(HPC)]
        )
        v_idx = np.concatenate(
            [np.arange(base + h * 3 * HD + 2 * HD, base + (h + 1) * 3 * HD)
             for h in range(HPC)]
        )
        in_maps.append({
            "xT": xT,
            "wqkv": np.ascontiguousarray(
                Wqkv[:, np.concatenate([qk_idx, v_idx])].astype(np.float16)
            ),
            "bqk": np.ascontiguousarray(bqkv[qk_idx].reshape(4 * HD, 1)),
            "bv": np.ascontiguousarray(
                bqkv[v_idx].reshape(1, 2 * HD).astype(np.float16)
            ),
            "cosT": cosT,
            "sinT": sinT_pm,
            "mask0": mask0,
            "wd": np.asarray(Wd, np.float32).astype(np.float16),
            "bdb": bdb,
        })
    return in_maps


def _get_nc():
    global _NC
    if _NC is None:
        _NC = _build_nc()
    return _NC


def _run(inputs, trace=False, tmpdir=None):
    from concourse.bass_utils import run_bass_kernel_spmd

    in_maps = _host_prep(**inputs)
    nc = _get_nc()
    res = run_bass_kernel_spmd(nc, in_maps, CORES, trace=trace, tmpdir=tmpdir)
    out = np.empty((M, D), np.float32)
    for c in CORES:
        yc = res.results[c]["y"]          # [512, D]: rows b0 then b1
        out[c * 256:(c + 1) * 256] = yc[:256]
        out[L + c * 256:L + (c + 1) * 256] = yc[256:]
    return out.reshape(B, L, D), res


def kernel(**inputs) -> np.ndarray:
    out, _ = _run(inputs)
    return out
